# revision 67
# baseline (speedup 1.0000x reference)
"""Distributed causal attention for TRN2 (8 NeuronCores).

Reference op (per core-external semantics):
    qkv = x @ w_qkv + b_qkv ; split into per-head q,k,v (16 heads, hd=64)
    causal softmax(q k^T / 8) v per head ; concat heads ; out = . @ w_proj + b_proj

Sharding: head-parallel attention (2 heads/core), AllToAll redistribution to
sequence-parallel for the output projection (each core owns S/8 query rows).

Schedule highlights:
  - Head 0's attention is interleaved with the qkv projection n-blocks
    (staggered one block back) so the scalar engine's exp stream starts
    early without ever stalling the tensor engine.
  - Each head's staged outputs are split into two 256-column windows with
    their own AllToAll; the output projection for window 0 overlaps the
    window-1 collective.
  - Staging/normalization DMAs issue from the idle gpsimd queue and the
    gather/store DMAs from the scalar queue to keep the sync queue free
    for bulk loads.
  - Softmax runs without max-subtraction (|score| < 4 at this scale);
    denominators come from a ones-column appended to V.

All matmuls run in bf16 (fp32 PSUM accumulation).

kernel(**inputs) takes the FULL fp32 inputs and returns the FULL fp32 output.
"""

import numpy as np
import ml_dtypes

import concourse.bacc as bacc
import concourse.bass as bass
import concourse.tile as tile
from concourse import masks, mybir
from concourse.bass_utils import run_bass_kernel_spmd

N_CORES = 8
D = 1024
H = 16
HD = 64
HPC = H // N_CORES          # heads per core = 2
MQKV = 3 * HPC * HD         # per-core qkv feature cols = 384

BF16 = mybir.dt.bfloat16
F32 = mybir.dt.float32
bf16 = ml_dtypes.bfloat16

# Bumping this changes the compiled executable's signature (a dummy input's
# shape encodes it), forcing a fresh compile + stage. Bump if a crashed run
# leaves a poisoned staged executable behind.
BUILD_SALT = 14


def build(S):
    QB = S // N_CORES        # query rows per core (A2A shard) = 512 for S=4096
    NQ = N_CORES             # number of q blocks == cores
    SKT = S // 128           # total sk tiles
    DIAG = QB // 128         # diagonal sk tiles per q block
    NPROJ = S // 512         # qkv-proj N blocks of 512
    G = 2                    # sk tiles per exp group
    WQ = QB // 2             # a2a window width (2 windows per head)

    nc = bacc.Bacc("TRN2", num_devices=N_CORES)

    xT = nc.declare_dram_parameter("xT", [D, S], BF16, isOutput=False)
    wqkv = nc.declare_dram_parameter("wqkv", [D, MQKV], BF16, isOutput=False)
    bqkv = nc.declare_dram_parameter("bqkv", [1, MQKV], BF16, isOutput=False)
    wproj = nc.declare_dram_parameter("wproj", [D, D], BF16, isOutput=False)
    bproj = nc.declare_dram_parameter("bproj", [1, D], BF16, isOutput=False)
    maskp = nc.declare_dram_parameter("mask", [QB, QB], BF16, isOutput=False)
    bqT = nc.declare_dram_parameter("bqT", [128, 3], F32, isOutput=False)
    salt = nc.declare_dram_parameter("salt", [1, BUILD_SALT], F32, isOutput=False)
    out_ext = nc.declare_dram_parameter("out", [QB, D], F32, isOutput=True)

    a2a_in = [
        [nc.dram_tensor(f"a2a_in{h}_{iw}", [NQ, HD, WQ], BF16) for iw in range(2)]
        for h in range(HPC)
    ]
    a2a_out = [
        [nc.dram_tensor(f"a2a_out{h}_{iw}", [NQ, HD, WQ], BF16) for iw in range(2)]
        for h in range(HPC)
    ]
    rden_dram = nc.dram_tensor("rden_dram", [HPC, NQ, QB], F32)
    # tiny scratch collectives keep the CC path warm so the real A2As
    # start at full interconnect bandwidth
    warm_in = [nc.dram_tensor(f"warm_in{i}", [NQ, 64], F32) for i in range(2)]
    warm_out = [nc.dram_tensor(f"warm_out{i}", [NQ, 64], F32) for i in range(2)]

    with tile.TileContext(nc) as tc:
        with (
            tc.tile_pool(name="singles", bufs=1) as singles,
            tc.tile_pool(name="dpool", bufs=1) as dpool,
            tc.tile_pool(name="rpool", bufs=2) as rpool,
            tc.tile_pool(name="opool", bufs=2) as opool,
            tc.tile_pool(name="nst", bufs=1) as nst,
            tc.tile_pool(name="qvp", bufs=2) as qvp,
            tc.tile_pool(name="bigx", bufs=1) as bigx,
            tc.tile_pool(name="ppool", bufs=8) as ppool,
            tc.tile_pool(name="upool", bufs=9) as upool,
            tc.tile_pool(name="ps1", bufs=2, space="PSUM") as ps1,
            tc.tile_pool(name="ps2", bufs=2, space="PSUM") as ps2,
            tc.tile_pool(name="ps3", bufs=1, space="PSUM") as ps3,
        ):
            # ---- load phase ----
            w_sb = singles.tile([128, 8, MQKV], BF16)
            nc.sync.dma_start(out=w_sb[:], in_=wqkv.rearrange("(a p) m -> p a m", p=128))
            bqT_sb = singles.tile([128, 3], F32)
            nc.sync.dma_start(out=bqT_sb[:], in_=bqT[:])
            mask_sb = singles.tile([128, DIAG, QB], BF16)
            nc.sync.dma_start(out=mask_sb[:], in_=maskp.rearrange("(d p) q -> p d q", p=128))
            ones_sb = singles.tile([1, 512], BF16)
            nc.vector.memset(ones_sb[:], 1.0)
            ident = singles.tile([128, 128], BF16)
            masks.make_identity(nc, ident[:])
            # x arrives in two 4-block waves sharing one 32KB slot; wave B's
            # DMAs wait (WAR via the pool) until wave A's proj blocks finish.
            NW = NPROJ // 2
            xT_r = xT.rearrange("(a p) s -> p a s", p=128)
            x_waves = []
            for wv in range(2):
                xw = bigx.tile([128, 8, NW * 512], BF16, tag="xw", name=f"x{wv}")
                for n in range(NW * wv, NW * (wv + 1)):
                    for a in range(8):
                        nc.sync.dma_start(
                            out=xw[:, a, 512 * (n - NW * wv):512 * (n - NW * wv + 1)],
                            in_=xT_r[:, a, 512 * n:512 * (n + 1)],
                        )
                x_waves.append(xw)
            bp_sb = singles.tile([1, D], BF16)
            nc.sync.dma_start(out=bp_sb[:], in_=bproj[:])
            salt_sb = singles.tile([1, BUILD_SALT], F32)
            nc.sync.dma_start(out=salt_sb[:], in_=salt[:])

            qkvT = singles.tile([128, 3, S], BF16)
            # v in natural [sk, hd] layout with ones column appended
            v_sb = singles.tile([128, SKT, 2 * (HD + 1)], BF16)
            nc.vector.memset(v_sb[:, :, HD:HD + 1], 1.0)
            nc.vector.memset(v_sb[:, :, 2 * HD + 1:2 * HD + 2], 1.0)

            # attention state (captured across emit calls)
            p_tiles_all = [[None] * NQ for _ in range(HPC)]
            un_tiles_all = [[None] * NQ for _ in range(HPC)]
            den_all = [dpool.tile([NQ, QB], F32, tag=f"den{h}", name=f"den{h}")
                       for h in range(HPC)]

            def col0(qb, t):
                """First unmasked q column of sk tile t in q block qb (columns
                below are fully causal-masked and skipped everywhere)."""
                nk = (qb + 1) * QB // 128
                d = t - (nk - DIAG)
                return 128 * d if d > 0 else 0

            def emit_qk_exp(h, qb):
                """QK^T + exp for q block qb of head h (S^T layout)."""
                nk = (qb + 1) * QB // 128  # causal sk tiles
                p_tiles = []
                for g0 in range(0, nk, G):
                    w = min(G, nk - g0)
                    ps = ps1.tile([128, 1024], F32, tag="ps1")
                    for j in range(w):
                        t = g0 + j
                        c0 = col0(qb, t)
                        nc.tensor.matmul(
                            ps[:, 512 * j + c0:512 * (j + 1)],
                            lhsT=qkvT[HD * h:HD * (h + 1), 1, 128 * t:128 * (t + 1)],
                            rhs=qkvT[HD * h:HD * (h + 1), 0, QB * qb + c0:QB * (qb + 1)],
                            start=True, stop=True,
                        )
                    pt = ppool.tile([128, 1024], BF16, tag="p")
                    if col0(qb, g0 + w - 1) == 0:
                        # no masked columns in this group: one batched exp
                        nc.scalar.activation(
                            pt[:, :512 * w], ps[:, :512 * w],
                            mybir.ActivationFunctionType.Exp, scale=0.125,
                        )
                    else:
                        for j in range(w):
                            c0 = col0(qb, g0 + j)
                            nc.scalar.activation(
                                pt[:, 512 * j + c0:512 * (j + 1)],
                                ps[:, 512 * j + c0:512 * (j + 1)],
                                mybir.ActivationFunctionType.Exp, scale=0.125,
                            )
                    p_tiles.append(pt)
                # causal mask on the diagonal tiles: only the 128-column
                # triangle straddle needs masking (earlier columns skipped,
                # later ones unmasked)
                for d in range(DIAG):
                    t = nk - DIAG + d
                    g0, j = divmod(t, G)
                    sl = slice(512 * j + 128 * d, 512 * j + 128 * (d + 1))
                    nc.vector.tensor_mul(
                        p_tiles[g0][:, sl], p_tiles[g0][:, sl],
                        mask_sb[:, d, 128 * d:128 * (d + 1)],
                    )
                p_tiles_all[h][qb] = p_tiles

            def emit_pv(h, qb):
                """PV: out^T (64 rows) + denominator (row 64) for q block qb."""
                nk = (qb + 1) * QB // 128
                p_tiles = p_tiles_all[h][qb]
                po = ps2.tile([HD + 1, QB], F32, tag="ps2")
                for t in range(nk):
                    g0, j = divmod(t, G)
                    c0 = col0(qb, t)
                    nc.tensor.matmul(
                        po[:, c0:],
                        lhsT=v_sb[:, t, (HD + 1) * h:(HD + 1) * (h + 1)],
                        rhs=p_tiles[g0][:, 512 * j + c0:512 * (j + 1)],
                        start=(t == 0), stop=(t == nk - 1),
                    )
                p_tiles_all[h][qb] = None
                un = upool.tile([HD + 1, QB], F32, tag="unorm")
                nc.vector.tensor_copy(un[:], po[:])
                un_tiles_all[h][qb] = un
                nc.sync.dma_start(out=den_all[h][qb:qb + 1, :], in_=un[HD:HD + 1, :])

            def emit_stage_and_a2a(h):
                """Normalize + stage head h, then fire its two window A2As.

                Batched: one rden store, one partition-broadcast load, one
                staging DMA per window — DMA-issue latency on the sync queue
                is what sits between the last PV and the A2A trigger.
                """
                den = den_all[h]
                rden = rpool.tile([NQ, QB], F32, tag="rden")
                nc.vector.reciprocal_approx_fast(rden[:], den[:])
                nc.sync.dma_start(out=rden_dram[h], in_=rden[:])
                bc = nst.tile([HD, NQ, QB], F32, tag="bc_all")
                src = bass.AP(
                    tensor=rden_dram,
                    offset=h * NQ * QB,
                    ap=[[0, HD], [QB, NQ], [1, QB]],
                )
                nc.sync.dma_start(out=bc[:], in_=src)
                stl = nst.tile([HD, NQ, QB], BF16, tag="st_all")
                for qb in range(NQ):
                    nc.vector.tensor_mul(
                        stl[:, qb, :], un_tiles_all[h][qb][0:HD, :], bc[:, qb, :]
                    )
                    un_tiles_all[h][qb] = None
                for iw in range(2):
                    nc.sync.dma_start(
                        out=a2a_in[h][iw][:].rearrange("g p s -> p g s"),
                        in_=stl[:, :, WQ * iw:WQ * (iw + 1)],
                    )
                for iw in range(2):
                    nc.gpsimd.collective_compute(
                        "AllToAll",
                        mybir.AluOpType.bypass,
                        replica_groups=[list(range(N_CORES))],
                        ins=[a2a_in[h][iw][:]],
                        outs=[a2a_out[h][iw][:]],
                    )

            # ---- qkv^T projection interleaved with head-0 attention ----
            # proj(n) computes qkvT features for seq block n; QK/exp for head0
            # q-block n-1 and PV for n-2 follow, so attention never waits on
            # the freshest projection and the tensor queue stays full.
            for n in range(NPROJ):
                xw = x_waves[n // NW]
                no = (n % NW) * 512
                for m in range(3):
                    ps = ps1.tile([128, 1024], F32, tag="ps1")
                    for a in range(8):
                        nc.tensor.matmul(
                            ps[:, 0:512],
                            lhsT=w_sb[:, a, 128 * m:128 * (m + 1)],
                            rhs=xw[:, a, no:no + 512],
                            start=(a == 0), stop=(a == 7),
                        )
                    # bias folds into the psum->sbuf copy as a per-partition add
                    if m < 2:
                        nc.vector.tensor_scalar_add(
                            qkvT[:, m, 512 * n:512 * (n + 1)], ps[:, 0:512],
                            bqT_sb[:, m:m + 1],
                        )
                    else:
                        # v: transpose to natural [sk, hd] layout
                        qv = qvp.tile([128, 512], BF16, tag="qv")
                        nc.vector.tensor_scalar_add(qv[:], ps[:, 0:512], bqT_sb[:, 2:3])
                        for tj in range(4):
                            t = 4 * n + tj
                            pt = ps3.tile([128, 128], BF16, tag="ps3")
                            nc.tensor.transpose(pt[:], qv[:, 128 * tj:128 * (tj + 1)], ident[:])
                            nc.vector.tensor_copy(v_sb[:, t, 0:HD], pt[:, 0:HD])
                            nc.vector.tensor_copy(v_sb[:, t, HD + 1:2 * HD + 1], pt[:, HD:2 * HD])
                # head-0 attention interleave, staggered one block back
                if n > 0:
                    emit_qk_exp(0, n - 1)
                if n > 1:
                    emit_pv(0, n - 2)
            emit_qk_exp(0, NPROJ - 1)
            emit_pv(0, NPROJ - 2)
            # warm the collective path shortly before head 0's A2As fire:
            # the scratch A2A is gated on a head-0 result so it runs late in
            # head-0's attention, not at kernel start
            nc.sync.dma_start(out=warm_in[0][0:1, :], in_=un_tiles_all[0][5][0:1, 0:64])
            nc.gpsimd.collective_compute(
                "AllToAll", mybir.AluOpType.bypass,
                replica_groups=[list(range(N_CORES))],
                ins=[warm_in[0][:]], outs=[warm_out[0][:]],
            )
            emit_pv(0, NPROJ - 1)
            emit_stage_and_a2a(0)

            # w_proj reuses the x slot (x is dead once the projection ends);
            # the load waits on the last proj matmul automatically.
            wp_sb = bigx.tile([128, 8, D], BF16, tag="xw", name="wp")
            nc.sync.dma_start(out=wp_sb[:], in_=wproj.rearrange("(a p) m -> p a m", p=128))

            # ---- head-1 attention ----
            # q blocks run in descending size order so the final block before
            # staging is the smallest; the warm-up A2A fires near the end
            for i, qb in enumerate(reversed(range(NQ))):
                emit_qk_exp(1, qb)
                emit_pv(1, qb)
                if qb == 2:
                    nc.sync.dma_start(
                        out=warm_in[1][0:1, :], in_=un_tiles_all[1][2][0:1, 0:64]
                    )
                    nc.gpsimd.collective_compute(
                        "AllToAll", mybir.AluOpType.bypass,
                        replica_groups=[list(range(N_CORES))],
                        ins=[warm_in[1][:]], outs=[warm_out[1][:]],
                    )
            emit_stage_and_a2a(1)

            # ---- output projection on local QB rows, per 256-row window ----
            # window iw's gather only needs a2a_out[*][iw], so part 0's
            # projection runs while the window-1 collective is in flight.
            for iw in range(2):
                ao = singles.tile([128, NQ, WQ], BF16, name=f"ao{iw}", tag=f"ao{iw}")
                for h in range(HPC):
                    # gpsimd queue: a gather here waits on its collective, and
                    # on the scalar/sync queues that wait blocks exp/staging
                    # work behind it (in-order queues)
                    nc.gpsimd.dma_start(
                        out=ao[HD * h:HD * (h + 1), :, :],
                        in_=a2a_out[h][iw][:].rearrange("g p s -> p g s"),
                    )
                for mi in range(WQ // 128):
                    mo = 128 * mi
                    ob = opool.tile([128, D], F32, tag="osb")
                    pf = ps1.tile([128, 1024], F32, tag="ps1")
                    for nh in range(2):
                        for g in range(8):
                            nc.tensor.matmul(
                                pf[:, 512 * nh:512 * (nh + 1)],
                                lhsT=ao[:, g, mo:mo + 128],
                                rhs=wp_sb[:, g, 512 * nh:512 * (nh + 1)],
                                start=(g == 0), stop=False,
                            )
                        nc.tensor.matmul(
                            pf[:, 512 * nh:512 * (nh + 1)],
                            lhsT=ones_sb[:, 0:128],
                            rhs=bp_sb[:, 512 * nh:512 * (nh + 1)],
                            start=False, stop=True,
                        )
                    nc.vector.tensor_copy(ob[:], pf[:])
                    nc.scalar.dma_start(
                        out=out_ext[WQ * iw + mo:WQ * iw + mo + 128, :], in_=ob[:]
                    )

    nc.compile()
    return nc


def make_in_maps(S, x, w_qkv, b_qkv, w_proj, b_proj):
    """Host-side sharding: returns per-core input dicts (bf16-cast)."""
    QB = S // N_CORES
    x2 = np.ascontiguousarray(x.reshape(S, D))
    xT = np.ascontiguousarray(x2.T).astype(bf16)
    wproj_b = w_proj.astype(bf16)
    bproj_b = b_proj.reshape(1, D).astype(bf16)
    i, j = np.indices((QB, QB))
    mask = (i <= j).astype(bf16)
    in_maps = []
    for c in range(N_CORES):
        cols = []
        bcols = []
        for part in range(3):  # q, k, v
            for hh in range(HPC):
                h = HPC * c + hh
                lo = part * D + HD * h
                cols.append(w_qkv[:, lo:lo + HD])
                bcols.append(b_qkv[lo:lo + HD])
        w_c = np.concatenate(cols, axis=1).astype(bf16)
        b_full = np.concatenate(bcols).astype(np.float32)
        b_c = b_full.reshape(1, MQKV).astype(bf16)
        bqT_c = np.ascontiguousarray(b_full.reshape(3, 128).T)  # [128, 3]
        in_maps.append({
            "xT": xT,
            "wqkv": np.ascontiguousarray(w_c),
            "bqkv": np.ascontiguousarray(b_c),
            "bqT": bqT_c,
            "wproj": wproj_b,
            "bproj": bproj_b,
            "mask": mask,
            "salt": np.zeros((1, BUILD_SALT), np.float32),
        })
    return in_maps


_CACHE = {}


def _get_nc(S):
    if S not in _CACHE:
        _CACHE[S] = build(S)
    return _CACHE[S]


def kernel(x, w_qkv, b_qkv, w_proj, b_proj, trace=False):
    x = np.asarray(x, dtype=np.float32)
    w_qkv = np.asarray(w_qkv, dtype=np.float32)
    b_qkv = np.asarray(b_qkv, dtype=np.float32)
    w_proj = np.asarray(w_proj, dtype=np.float32)
    b_proj = np.asarray(b_proj, dtype=np.float32)
    B, S, _ = x.shape
    nc = _get_nc(S)
    in_maps = make_in_maps(S, x, w_qkv, b_qkv, w_proj, b_proj)
    res = run_bass_kernel_spmd(nc, in_maps, core_ids=list(range(N_CORES)), trace=trace)
    QB = S // N_CORES
    out = np.empty((S, D), dtype=np.float32)
    for c in range(N_CORES):
        out[QB * c:QB * (c + 1)] = res.results[c]["out"]
    if trace:
        kernel.last_exec_time_ns = res.exec_time_ns
        kernel.last_result = res
    return out.reshape(B, S, D)
